# revision 26
# baseline (speedup 1.0000x reference)
"""Trainium2 Bass kernel for nn_MultiHeadAttention_49770081026139.

Multi-head attention with an edge tensor:
    qh = (q @ Wq + bq) * d^-0.5 ; kh = k @ Wk + bk ; vh = v @ Wv + bv
    eh = e @ We + be                      (b, i, j, H) -> heads (b, h, i, j, d)
    qk_e[b,h,i,j,d] = qh[b,h,i,d] * kh[b,h,j,d] * eh[b,h,i,j,d]
    w  = qk_e.sum(d) + attn_bias ; a = softmax_j(w)
    x  = (a @ vh) combined-heads @ Wo + bo            -> (b, n, H)
    w_out = qk_e rearranged (b,i,j,(h d)) @ Weo + beo -> (b, n, n, H)

Sharding: 8 cores = 4 batches x 2 halves of the i axis. Fully data
parallel (softmax is over j, kept whole per core) - no collectives.

On-device layout is "feature major": channels live on SBUF partitions
(4 chunks of 128) so every linear layer is a PE matmul with the weight
chunk [cin,cout] stationary, and the per-i broadcast of qh becomes a
per-partition tensor_scalar. The e tensor is ingested already
transposed AND cast: the fp32 words in HBM are viewed as bf16 pairs
and the high halves are DMA-transposed (xbar) straight into SBUF,
which is bf16 truncation of e. Big matmuls run in bf16 (1 cyc/row)
with fp32 PSUM accumulation, two i-rows at a time (N=512 moving).
w_out is stored bf16 and upcast on the host; bo is applied on device,
beo on the host (both zero in this problem).
"""

import numpy as np
import ml_dtypes

BF = ml_dtypes.bfloat16

B, NTOK, HID = 4, 256, 512
NHEAD, DHEAD = 16, 32
ILOC = 128          # i rows per core
NJ = 256            # full j per core
NCH = 4             # channel chunks (512 / 128)
NCORES = 8

_last_results = None  # stash of BassKernelResults for test harness introspection


def _split_multi_waits(mod):
    """This container's walrus accepts at most one sync-wait per instruction;
    Tile's scheduler embeds several. Hoist extras into standalone
    EventSemaphore waits on the same engine immediately before the
    instruction (same-engine program order makes this equivalent)."""
    for fn in mod["functions"]:
        for blk in fn["blocks"]:
            out = []
            for inst in blk["instructions"]:
                si = inst.get("sync_info")
                waits = (si or {}).get("on_wait") or []
                limit = 0 if inst.get("opcode") == "DMACopy" else 1
                if len(waits) > limit:
                    keep = waits[len(waits) - limit:]
                    hoist = waits[:len(waits) - limit]
                    for k, w in enumerate(hoist):
                        out.append({
                            "debug": inst.get("debug", 0),
                            "engine": inst["engine"],
                            "ins": [], "outs": [],
                            "name": f"{inst['name']}.w{k}",
                            "opcode": "EventSemaphore",
                            "sync_info": {"on_update": [], "on_wait": [w]},
                        })
                    si["on_wait"] = keep
                out.append(inst)
            blk["instructions"] = out
    return mod


def _patch_json_serialization(nc):
    import orjson

    orig = nc.to_json_bytes

    def patched():
        return orjson.dumps(_split_multi_waits(orjson.loads(orig())))

    nc.to_json_bytes = patched
    return nc


def build_nc(n_i=ILOC, e_transpose="dma"):
    """Build the per-core Bass program (SPMD: same program, different data)."""
    from contextlib import ExitStack

    import concourse.bass as bass
    import concourse.mybir as mybir
    import concourse.tile as tile

    f32 = mybir.dt.float32
    bf = mybir.dt.bfloat16
    AX = mybir.AxisListType
    ALU = mybir.AluOpType
    ACTF = mybir.ActivationFunctionType

    assert n_i % 8 == 0
    nblk8 = n_i // 8

    nc = bass.Bass()

    e_d = nc.dram_tensor("e_loc", [n_i, NJ, HID], f32, kind="ExternalInput")
    q_d = nc.dram_tensor("q_loc", [ILOC, HID], f32, kind="ExternalInput")
    k_d = nc.dram_tensor("k_loc", [NJ, HID], f32, kind="ExternalInput")
    v_d = nc.dram_tensor("v_loc", [NJ, HID], f32, kind="ExternalInput")
    bias_d = nc.dram_tensor("bias_loc", [nblk8, 128, 2, NJ], f32, kind="ExternalInput")
    w_names = ["wq", "wk", "wv", "wo", "we", "weo"]
    w_d = {n: nc.dram_tensor(n, [128, NCH, HID], bf, kind="ExternalInput") for n in w_names}
    ident_d = nc.dram_tensor("ident", [128, 128], bf, kind="ExternalInput")
    blk_d = nc.dram_tensor("blk", [128, NCH * NHEAD], bf, kind="ExternalInput")
    brows_d = nc.dram_tensor("brows", [1, 4, HID], bf, kind="ExternalInput")
    becol_d = nc.dram_tensor("becol", [128, NCH], f32, kind="ExternalInput")

    wout_d = nc.dram_tensor("wout_loc", [n_i, NJ, HID], bf, kind="ExternalOutput")
    x_d = nc.dram_tensor("x_loc", [ILOC, HID], f32, kind="ExternalOutput")

    with tile.TileContext(nc) as tc, ExitStack() as ctx:
        consts = ctx.enter_context(tc.tile_pool(name="consts", bufs=1))
        prep = ctx.enter_context(tc.tile_pool(name="prep", bufs=1))
        p_ebf = ctx.enter_context(tc.tile_pool(name="p_ebf", bufs=3))
        p_eT = ctx.enter_context(tc.tile_pool(name="p_eT", bufs=3))
        p_ehb = ctx.enter_context(tc.tile_pool(name="p_ehb", bufs=2))
        p_qk = ctx.enter_context(tc.tile_pool(name="p_qk", bufs=2))
        p_wout = ctx.enter_context(tc.tile_pool(name="p_wout", bufs=3))
        p_bias = ctx.enter_context(tc.tile_pool(name="p_bias", bufs=2))
        p_sm = ctx.enter_context(tc.tile_pool(name="p_sm", bufs=2))
        p_small = ctx.enter_context(tc.tile_pool(name="p_small", bufs=4))
        psEH = ctx.enter_context(tc.tile_pool(name="psEH", bufs=2, space="PSUM"))
        psW = ctx.enter_context(tc.tile_pool(name="psW", bufs=2, space="PSUM"))
        psWO = ctx.enter_context(tc.tile_pool(name="psWO", bufs=3, space="PSUM"))
        psX = ctx.enter_context(tc.tile_pool(name="psX", bufs=1, space="PSUM"))

        # ---- constants -------------------------------------------------
        ident_sb = consts.tile([128, 128], bf)
        nc.sync.dma_start(ident_sb, ident_d[:, :])
        blk_sb = consts.tile([128, NCH * NHEAD], bf)
        nc.sync.dma_start(blk_sb, blk_d[:, :])
        brows_sb = consts.tile([1, 4, HID], bf)
        nc.sync.dma_start(brows_sb, brows_d[:, :, :])
        becol_sb = consts.tile([128, NCH], f32)
        nc.sync.dma_start(becol_sb, becol_d[:, :])
        ones_sb = consts.tile([1, NJ], bf)
        nc.vector.memset(ones_sb, 1.0)
        w_sbs = {}
        for n in w_names:
            w_sbs[n] = consts.tile([128, NCH, HID], bf, name=n, tag=n)
            nc.sync.dma_start(w_sbs[n], w_d[n][:, :, :])

        # ---- prep: project q, k, v ------------------------------------
        # q -> qhT_sb [cout%128, (chunk, i)] fp32 (per-partition scalars)
        q_sb = prep.tile([128, HID], f32)
        nc.sync.dma_start(q_sb, q_d[:, :])
        q_bf = prep.tile([128, HID], bf)
        nc.scalar.copy(q_bf, q_sb)
        ps_t = psEH.tile([128, NCH, 128], bf, tag="eh")
        for r in range(NCH):
            nc.tensor.transpose(ps_t[:, r, :], q_bf[:, r * 128:(r + 1) * 128], ident_sb)
        qT_bf = prep.tile([128, NCH, 128], bf)
        nc.vector.tensor_copy(qT_bf, ps_t)
        ps_qh = psEH.tile([128, NCH, 128], f32, tag="eh")
        for ro in range(NCH):
            for ri in range(NCH):
                nc.tensor.matmul(ps_qh[:, ro, :], w_sbs["wq"][:, ri, ro * 128:(ro + 1) * 128],
                                 qT_bf[:, ri, :], start=(ri == 0), stop=False)
            nc.tensor.matmul(ps_qh[:, ro, :], brows_sb[0:1, 0, ro * 128:(ro + 1) * 128],
                             ones_sb[0:1, 0:128], start=False, stop=True)
        qhT_sb = consts.tile([128, NCH, 128], f32)
        nc.vector.tensor_copy(qhT_sb, ps_qh)

        # k -> khT2_bf [cout%128, (chunk, ii, j)] bf16 (kh duplicated over ii)
        k_sb = prep.tile([128, 2, HID], f32)
        nc.sync.dma_start(k_sb, k_d.rearrange("(jh p) c -> p jh c", p=128))
        k_bf = prep.tile([128, 2, HID], bf)
        nc.scalar.copy(k_bf, k_sb)
        ps_tk = psEH.tile([128, NCH, NJ], bf, tag="eh")
        for jh in range(2):
            for r in range(NCH):
                nc.tensor.transpose(ps_tk[:, r, jh * 128:(jh + 1) * 128],
                                    k_bf[:, jh, r * 128:(r + 1) * 128], ident_sb)
        kT_bf = prep.tile([128, NCH, NJ], bf)
        nc.vector.tensor_copy(kT_bf, ps_tk)
        khT2_bf = consts.tile([128, NCH, 2, NJ], bf)
        for ro in range(NCH):
            ps_kh = psEH.tile([128, NJ], f32, tag="eh", name="ps_kh")
            for ri in range(NCH):
                nc.tensor.matmul(ps_kh, w_sbs["wk"][:, ri, ro * 128:(ro + 1) * 128],
                                 kT_bf[:, ri, :], start=(ri == 0), stop=False)
            nc.tensor.matmul(ps_kh, brows_sb[0:1, 1, ro * 128:(ro + 1) * 128],
                             ones_sb[0:1, :], start=False, stop=True)
            for ii in range(2):
                nc.vector.tensor_copy(khT2_bf[:, ro, ii, :], ps_kh)

        # v -> vh_sb [j%128, (jh, cout)] bf16 (natural layout, matvec lhsT)
        v_sb = prep.tile([128, 2, HID], f32)
        nc.sync.dma_start(v_sb, v_d.rearrange("(jh p) c -> p jh c", p=128))
        v_bf = prep.tile([128, 2, HID], bf)
        nc.scalar.copy(v_bf, v_sb)
        ps_tv = psEH.tile([128, NCH, NJ], bf, tag="eh")
        for jh in range(2):
            for r in range(NCH):
                nc.tensor.transpose(ps_tv[:, r, jh * 128:(jh + 1) * 128],
                                    v_bf[:, jh, r * 128:(r + 1) * 128], ident_sb)
        vT_bf = prep.tile([128, NCH, NJ], bf)
        nc.vector.tensor_copy(vT_bf, ps_tv)
        vh_sb = consts.tile([128, 2, HID], bf)
        for jh in range(2):
            ps_vh = psEH.tile([128, HID], f32, tag="eh")
            for ri in range(NCH):
                nc.tensor.matmul(ps_vh, vT_bf[:, ri, jh * 128:(jh + 1) * 128],
                                 w_sbs["wv"][:, ri, :], start=(ri == 0), stop=False)
            nc.tensor.matmul(ps_vh, ones_sb[0:1, 0:128], brows_sb[0:1, 2, :],
                             start=False, stop=True)
            nc.vector.tensor_copy(vh_sb[:, jh, :], ps_vh)

        # attention-output accumulator: xT [cout%128=(h%4)*32+d, (chunk=h//4, i)]
        ps_x = psX.tile([128, NCH, ILOC], f32)
        nc.vector.memset(ps_x, 0.0)

        # beq[c, i] = be[c] * qh[c, i]  (activation bias for the fused scale)
        beq_sb = consts.tile([128, NCH, ILOC], f32)
        for ro in range(NCH):
            nc.vector.tensor_scalar_mul(beq_sb[:, ro, :], qhT_sb[:, ro, :],
                                        becol_sb[:, ro:ro + 1])

        # ---- main loop over pairs of i --------------------------------
        ps_w4 = None
        bias_sb = None
        for p in range(n_i // 2):
            i0 = 2 * p
            p4, blk8 = p % 4, p // 4

            # load both rows, casting fp32->bf16 in the SWDGE DMA
            e_bf = p_ebf.tile([128, 2, 2, HID], bf)
            nc.gpsimd.dma_start(
                e_bf, e_d[i0:i0 + 2].rearrange("ii (jh pp) c -> pp ii jh c", pp=128))

            # transpose to feature-major eT [c%128, (chunk, ii, j)]
            eT = p_eT.tile([128, NCH, 2, NJ], bf)
            if e_transpose == "dma":
                for ii in range(2):
                    for rc in range(NCH):
                        for jh in range(2):
                            nc.sync.dma_start(
                                eT[:, rc, ii, jh * 128:(jh + 1) * 128],
                                e_bf[:, ii, jh, rc * 128:(rc + 1) * 128],
                                transpose=True)
            else:
                ps_eT = psWO.tile([128, NCH, 2, NJ], bf, tag="wo")
                for ii in range(2):
                    for jh in range(2):
                        for rc in range(NCH):
                            nc.tensor.transpose(
                                ps_eT[:, rc, ii, jh * 128:(jh + 1) * 128],
                                e_bf[:, ii, jh, rc * 128:(rc + 1) * 128], ident_sb)
                nc.scalar.copy(eT, ps_eT)

            # mm1 per cout chunk (fp32 accum), then ACT folds be & qh per row:
            # ehb = Identity(ehT * qh + be*qh)
            ehb_bf = p_ehb.tile([128, NCH, 2, NJ], bf)
            for ro in range(NCH):
                ps_eh2 = psEH.tile([128, 2, NJ], f32, tag="eh")
                for ri in range(NCH):
                    nc.tensor.matmul(ps_eh2, w_sbs["we"][:, ri, ro * 128:(ro + 1) * 128],
                                     eT[:, ri, :, :], start=(ri == 0), stop=(ri == NCH - 1))
                for ii in range(2):
                    nc.scalar.activation(ehb_bf[:, ro, ii, :], ps_eh2[:, ii, :],
                                         ACTF.Identity,
                                         bias=beq_sb[:, ro, i0 + ii:i0 + ii + 1],
                                         scale=qhT_sb[:, ro, i0 + ii:i0 + ii + 1])
            # qk_e^T = ehb * kh^T   (one op over the whole pair)
            qk_bf = p_qk.tile([128, NCH, 2, NJ], bf)
            nc.vector.tensor_mul(qk_bf, ehb_bf, khT2_bf)

            # logits for both rows: partitions [32*p4, +16), free (ii, j)
            if p4 == 0:
                ps_w4 = psW.tile([128, 2, NJ], f32)
                nc.vector.memset(ps_w4, 0.0)  # init head-pad partitions
                bias_sb = p_bias.tile([128, 2, NJ], f32)
                nc.sync.dma_start(bias_sb, bias_d[blk8])
            for qq in range(NCH):
                nc.tensor.matmul(ps_w4[32 * p4:32 * p4 + 16, :, :],
                                 blk_sb[:, qq * 16:(qq + 1) * 16], qk_bf[:, qq, :, :],
                                 start=(qq == 0), stop=(qq == NCH - 1),
                                 tile_position=(0, 32 * p4))

            # mm2: w_out natural [j, cout]; one PSUM bank per (ii, jh)
            for ii in range(2):
                wout_sb = p_wout.tile([128, 2, HID], bf)
                for jh in range(2):
                    ps_wo = psWO.tile([128, HID], f32, tag="wo")
                    for r in range(NCH):
                        nc.tensor.matmul(ps_wo, qk_bf[:, r, ii, jh * 128:(jh + 1) * 128],
                                         w_sbs["weo"][:, r, :], start=(r == 0),
                                         stop=(r == NCH - 1))
                    if jh == 0:
                        nc.vector.tensor_copy(wout_sb[:, jh, :], ps_wo)
                    else:
                        nc.scalar.copy(wout_sb[:, jh, :], ps_wo)
                nc.sync.dma_start(
                    wout_d[i0 + ii].rearrange("(jh pp) c -> pp jh c", pp=128), wout_sb)

            # softmax + attention once per 4 pairs (8 rows)
            if p4 == 3:
                w_sm = p_sm.tile([128, 2, NJ], f32, tag="w_sm")
                nc.vector.tensor_add(w_sm, ps_w4, bias_sb)
                negmax = p_small.tile([128, 2], f32, tag="negmax")
                nc.vector.tensor_reduce(negmax, w_sm, axis=AX.X, op=ALU.max, negate=True)
                a_bf = p_sm.tile([128, 2, NJ], bf, tag="a_bf")
                for ii in range(2):
                    a_f = p_sm.tile([128, NJ], f32, tag="a_f")
                    sums = p_small.tile([128, 1], f32, tag="sums")
                    nc.scalar.activation(a_f, w_sm[:, ii, :], ACTF.Exp,
                                         bias=negmax[:, ii:ii + 1], scale=1.0,
                                         accum_out=sums[:, 0:1])
                    rinv = p_small.tile([128, 1], f32, tag="rinv")
                    nc.vector.reciprocal(rinv, sums)
                    nc.vector.tensor_scalar_mul(a_bf[:, ii, :], a_f, rinv[:, 0:1])

                # transpose a -> [j%128, (jh, ii, 4p x 32)]
                ps_aT = psWO.tile([128, 2, 2, 4, 32], bf, tag="wo")
                for jh in range(2):
                    for ii in range(2):
                        nc.tensor.transpose(ps_aT[:, jh, ii, :, :],
                                            a_bf[:, ii, jh * 128:(jh + 1) * 128],
                                            ident_sb)
                # permute to (pp, ii) so matvec rhs/out flatten identically
                aT_sb = p_sm.tile([128, 2, 4, 2, 32], bf, tag="aT_sb")
                nc.vector.tensor_copy(
                    aT_sb, ps_aT.rearrange("p jh ii pp hh -> p jh pp ii hh"))

                # x[h] += a_h @ vh_h for the 8 rows of this block
                # rhs free order (pp, ii) matches i = 8*blk8 + 2*pp + ii
                xv = ps_x.rearrange("q c (b pp ii) -> q c b pp ii", b=ILOC // 8, ii=2)
                for h in range(NHEAD):
                    out_sl = xv[32 * (h % 4):32 * (h % 4) + 32, h // 4, blk8, :, :]
                    for jh in range(2):
                        nc.tensor.matmul(out_sl, vh_sb[:, jh, h * 32:(h + 1) * 32],
                                         aT_sb[:, jh, :, :, h], start=(jh == 0),
                                         stop=(jh == 1),
                                         tile_position=(0, 32 * (h % 4)))

        # ---- epilogue: x out-projection -------------------------------
        xT_bf = prep.tile([128, NCH, ILOC], bf)
        nc.vector.tensor_copy(xT_bf, ps_x)
        ps_xo = psEH.tile([128, HID], f32, tag="eh")
        for ri in range(NCH):
            nc.tensor.matmul(ps_xo, xT_bf[:, ri, :], w_sbs["wo"][:, ri, :],
                             start=(ri == 0), stop=False)
        nc.tensor.matmul(ps_xo, ones_sb[0:1, 0:128], brows_sb[0:1, 3, :],
                         start=False, stop=True)
        x_sb = prep.tile([128, HID], f32)
        nc.vector.tensor_copy(x_sb, ps_xo)
        nc.sync.dma_start(x_d[:, :], x_sb)

    return _patch_json_serialization(nc)


def make_host_inputs(q, k, v, e, attn_bias, Wq, bq, Wk, bk, Wv, bv, We, be,
                     Wo, bo, Weo, beo, n_i=ILOC):
    """Prepare per-core input maps (host-side layout only: slicing, weight
    chunking, bf16 cast of the small weight tensors)."""
    scale = DHEAD ** -0.5

    def chunkw(w):
        w = np.asarray(w, np.float32)
        return np.ascontiguousarray(w.reshape(NCH, 128, HID).transpose(1, 0, 2)).astype(BF)

    weights = {
        "wq": chunkw(np.asarray(Wq) * scale), "wk": chunkw(Wk), "wv": chunkw(Wv),
        "wo": chunkw(Wo), "we": chunkw(We), "weo": chunkw(Weo),
    }
    brows = np.zeros((1, 4, HID), BF)
    brows[0, 0] = (np.asarray(bq, np.float32) * scale).astype(BF)
    brows[0, 1] = np.asarray(bk, np.float32).astype(BF)
    brows[0, 2] = np.asarray(bv, np.float32).astype(BF)
    brows[0, 3] = np.asarray(bo, np.float32).astype(BF)
    becol = np.ascontiguousarray(np.asarray(be, np.float32).reshape(NCH, 128).T)
    ident = np.eye(128, dtype=BF)
    blk = np.zeros((128, NCH * NHEAD), BF)
    for qq in range(NCH):
        for cc in range(128):
            blk[cc, qq * NHEAD + 4 * qq + cc // 32] = 1.0

    nblk8 = n_i // 8
    in_maps = []
    for c in range(NCORES):
        b, i0 = c // 2, (c % 2) * ILOC
        # bias_re[blk8, 32*p4 + h, ii, j] = attn_bias[b, h, i0 + 8*blk8 + 2*p4 + ii, j]
        ab = np.asarray(attn_bias[b, :, i0:i0 + n_i, :], np.float32)  # [16, n_i, 256]
        bias_re = np.zeros((nblk8, 4, 32, 2, NJ), np.float32)
        # ab -> [nblk8, 4p4, 2ii, 16h, j]
        ab_r = ab.transpose(1, 0, 2).reshape(nblk8, 4, 2, NHEAD, NJ)
        bias_re[:, :, :NHEAD, :, :] = ab_r.transpose(0, 1, 3, 2, 4)
        m = {
            "e_loc": np.asarray(e[b, i0:i0 + n_i], np.float32),
            "q_loc": np.asarray(q[b, i0:i0 + ILOC], np.float32),
            "k_loc": np.asarray(k[b], np.float32),
            "v_loc": np.asarray(v[b], np.float32),
            "bias_loc": bias_re.reshape(nblk8, 128, 2, NJ),
            "ident": ident, "blk": blk, "brows": brows, "becol": becol,
        }
        m.update(weights)
        in_maps.append(m)
    return in_maps


def kernel(q, k, v, e, attn_bias, num_heads, Wq, bq, Wk, bk, Wv, bv, We, be,
           Wo, bo, Weo, beo):
    global _last_results
    import os

    from concourse.bass_utils import run_bass_kernel_spmd

    assert int(num_heads) == NHEAD

    in_maps = make_host_inputs(q, k, v, e, attn_bias, Wq, bq, Wk, bk, Wv, bv,
                               We, be, Wo, bo, Weo, beo)
    nc = build_nc()
    trace = os.environ.get("KERNEL_TRACE", "0") not in ("", "0")
    res = run_bass_kernel_spmd(nc, in_maps, list(range(NCORES)), trace=trace)
    _last_results = res

    x = np.empty((B, NTOK, HID), np.float32)
    wout = np.empty((B, NTOK, NJ, HID), np.float32)
    for c in range(NCORES):
        b, i0 = c // 2, (c % 2) * ILOC
        x[b, i0:i0 + ILOC] = res.results[c]["x_loc"]
        wout[b, i0:i0 + ILOC] = res.results[c]["wout_loc"].astype(np.float32)
    beo_a = np.asarray(beo, np.float32)
    if np.any(beo_a):
        wout += beo_a  # additive output-side bias, applied on host
    return (x, wout)
